# revision 16
# baseline (speedup 1.0000x reference)
"""Trainium2 Bass kernel for nn_Decoder — dual-column GRU decoder.

Design ("SG8v2") — zero-gap PE schedule
---------------------------------------
Data-parallel over batch: 8 cores x 8 batch rows -> 32768 columns per core.
A "dchunk" is 1024 columns stored as a [128, 512] tile — columns 0:512 on
partitions 0:64, columns 512:1024 on partitions 64:128.  Gate matmuls use
block-diagonal lhsT [128,128] = diag(W.T, W.T).

A supergroup of 8 dchunks = 4 pairs is pipelined per step.  The TRN2 tensor
engine only reaches 2.4 GHz after 3us of *continuous* execution, so the
whole schedule is built to never gap the PE:

- gate psum (az/ar): one shared tag, 3 banks, per-dchunk [128,512] tiles
  rotating ar-d0, ar-d1, az-d0, az-d1; the WAR distance to each sigmoid is
  >= 2 passes.
- hn and np live in SEPARATE per-pair [128,1024] allocations (tag P,
  bufs=2, 4 banks): fn/xn/eye never wait on the v-STT's read of hn.
- preds accumulate in ONE psum bank for all 8 dchunks
  (row = 32*pair + 16*(d%2) + 2*t8 + half) using two lhsT variants
  (WO32A/WO32B); flushes are lagged TWO pair-blocks; the h'-update DVE ops
  are lagged ONE pair-block so the vp STTs never queue behind them.
- epoch evacuation is issued at the head of pair-2's block of the next
  window, overlapping the new window's first flushes.

Per step t and pair (h [128,1024] = dchunks d0|d1):
  ar   = blockdiag(Ar.T) @ h + X_r @ xrows      (Ar = Wr + Gr0*wo' fold)
  az   = blockdiag(Az.T) @ h + X_z @ xrows
  r, z = sigmoid(ar), sigmoid(az)               (ACT, [128,512] per dchunk)
  hn   = blockdiag(Wn.T) @ h                    (per-pair PSUM)
  v    = (hn + b_hhn) * r                       (DVE STT per dchunk)
  np   = blockdiag(Fn.T)@h + X_n @ xrows        (separate per-pair PSUM)
  n    = tanh(np + v)                           (DVE add, then ACT
                                                 [128,1024]; no EYE pass)
  h'   = n + z*(h - n)                          (3 DVE ops, pair-wide,
                                                 issued in the next block)
  pred = wo-row lhsT @ h'  (+bo at evacuation), flushed two blocks later

x-rows per (dchunk, t): [12, 512] = per half [xt(3); xn(t=0); 1; bo-step].
"""

import os

import numpy as np

import concourse.bass as bass
import concourse.mybir as mybir
import concourse.tile as tile
from concourse import bacc
from concourse.bass_utils import run_bass_kernel_spmd

F32 = mybir.dt.float32
BF16 = mybir.dt.bfloat16
FP8 = mybir.dt.float8e4
AF = mybir.ActivationFunctionType
ALU = mybir.AluOpType

B, T_HIST, T_FC, C, F_IN, HID = 64, 24, 48, 4096, 8, 64
N_CORES = 8
B_LOC = B // N_CORES
NCOLS = B_LOC * C            # 32768 columns per core
DCH = 1024                   # columns per dchunk (dual-packed)
NDCH = NCOLS // DCH          # 32 dchunks
SG = 8                       # dchunks per supergroup
NSG = NDCH // SG             # 4 supergroups
NEP = T_FC // 8              # 6 pred epochs of 8 steps
XPF = 4                      # x prefetch lead (steps)

_BUILT = {}
LAST_RESULTS = None

W128 = ["AZ1", "AR1", "AZ0", "AR0", "WN", "FN", "EYE"]


def _build():
    if "nc" in _BUILT:
        return _BUILT["nc"]

    nc = bacc.Bacc("TRN2", target_bir_lowering=False, debug=False,
                   num_devices=N_CORES)

    d_ht = nc.dram_tensor("HT", [NDCH, 128, 512], BF16,
                          kind="ExternalInput").ap()
    d_xt = nc.dram_tensor("XT", [NDCH, T_FC, 12, 512], BF16,
                          kind="ExternalInput").ap()
    d_w = {}
    for name in W128:
        d_w[name] = nc.dram_tensor(name, [128, 128], BF16,
                                   kind="ExternalInput").ap()
    for name in ("XZ", "XR", "XN"):
        d_w[name] = nc.dram_tensor(name, [12, 128], BF16,
                                   kind="ExternalInput").ap()
    d_w["WO32A"] = nc.dram_tensor("WO32A", [128, 256], BF16,
                                  kind="ExternalInput").ap()
    d_w["WO32B"] = nc.dram_tensor("WO32B", [128, 256], BF16,
                                  kind="ExternalInput").ap()
    d_w["BNH"] = nc.dram_tensor("BNH", [128, 1], F32,
                                kind="ExternalInput").ap()
    d_out = nc.dram_tensor("OUT", [NSG, 128, 512 * NEP], BF16,
                           kind="ExternalOutput").ap()

    with tile.TileContext(nc) as tc:
        with (
            tc.tile_pool(name="wpool", bufs=1) as wpool,
            tc.tile_pool(name="xpool", bufs=1) as xpool,
            tc.tile_pool(name="hpool", bufs=1) as hpool,
            tc.tile_pool(name="tpool", bufs=1) as tpool,
            tc.tile_pool(name="opool", bufs=1) as opool,
            tc.tile_pool(name="pspool", bufs=1, space="PSUM") as pspool,
        ):
            w = {}
            for name, ap in d_w.items():
                wt = wpool.tile(list(ap.shape), ap.dtype, name=f"w_{name}")
                nc.gpsimd.dma_start(wt[:], ap[:])
                w[name] = wt

            hnext = None
            xts = {}

            def load_x(sgi, t0):
                for dp in range(SG):
                    xt = xpool.tile([12, 512], BF16, tag=f"x{dp}",
                                    bufs=XPF + 5, name="xt")
                    nc.sync.dma_start(xt[:], d_xt[sgi * SG + dp, t0])
                    xts[(sgi * SG + dp, t0)] = xt

            def load_h(sgi):
                hh = {}
                for pr in range(4):
                    ht = hpool.tile([128, DCH], BF16, tag=f"h{pr}", bufs=3,
                                    name="ht")
                    for dd in range(2):
                        d = sgi * SG + 2 * pr + dd
                        nc.sync.dma_start(ht[:, 512 * dd:512 * (dd + 1)],
                                          d_ht[d])
                    hh[pr] = ht
                return hh

            for sg in range(NSG):
                dbase = sg * SG
                hcur = hnext if hnext is not None else load_h(sg)
                hnext = None

                for t0 in range(XPF):
                    if (dbase, t0) not in xts:
                        load_x(sg, t0)

                outst = opool.tile([128, 512 * NEP], BF16, tag="ost",
                                   bufs=2, name="outstage")

                state = {"pend": [], "hprev": None, "pred": None,
                         "pred_done": None, "blk": 0}

                def flush_blocks(blks):
                    # row map: dl = 2*pr+ddl -> block 32*(dl%4), variant
                    # dl//4 — both dchunks of a pair share one WO32 variant
                    for blk in blks:
                        for (pr, ddl, htile, t8) in blk:
                            dl = 2 * pr + ddl
                            lhs = w["WO32A"] if dl < 4 else w["WO32B"]
                            nc.tensor.matmul(
                                state["pred"][32 * (dl % 4):
                                              32 * (dl % 4) + 32, :],
                                lhs[:, 32 * t8:32 * t8 + 32],
                                htile[:, 512 * ddl:512 * (ddl + 1)],
                                start=(t8 == 0 and dl < 4),
                                stop=(t8 == 7 and dl >= 4),
                                tile_position=(0, 32 * (dl % 4)))

                def issue_hprime():
                    hp = state["hprev"]
                    if hp is None:
                        return
                    ht_o, nt_o, zs_o, hnew_o = hp
                    hm = tpool.tile([128, DCH], BF16, tag="hm", bufs=2,
                                    name="hm")
                    nc.vector.tensor_tensor(hm[:], ht_o[:], nt_o[:],
                                            op=ALU.subtract)
                    ztt = tpool.tile([128, DCH], BF16, tag="zt", bufs=2,
                                     name="ztt")
                    nc.vector.tensor_tensor(ztt[:], zs_o[:], hm[:],
                                            op=ALU.mult)
                    nc.vector.tensor_tensor(hnew_o[:], nt_o[:], ztt[:],
                                            op=ALU.add)
                    state["hprev"] = None

                for t in range(T_FC):
                    if t == T_FC - 2 and sg + 1 < NSG:
                        hnext = load_h(sg + 1)
                        for t0 in range(XPF):
                            load_x(sg + 1, t0)
                    tp = t + XPF
                    if tp < T_FC:
                        load_x(sg, tp)

                    for pr in range(4):
                        ht = hcur[pr]
                        xx = {0: xts.pop((dbase + 2 * pr, t)),
                              1: xts.pop((dbase + 2 * pr + 1, t))}
                        azw = w["AZ1"] if t > 0 else w["AZ0"]
                        arw = w["AR1"] if t > 0 else w["AR0"]

                        # pred-window bookkeeping: at the 8-step boundary the
                        # evac of the finished window is issued at the head of
                        # pair-2's block, then the new pred bank is allocated.
                        if pr == 2 and t % 8 == 0:
                            if state["pred"] is not None:
                                state["pred_done"] = (state["pred"],
                                                      t // 8 - 1)
                            state["pred"] = pspool.tile(
                                [128, 512], F32, tag="pred", bufs=1,
                                name="predp")

                        # --- gate matmuls (3-bank rotation ar0,ar1,az0,az1)
                        g = {}
                        for nmr, dd in (("r", 0), ("r", 1)):
                            g[(nmr, dd)] = pspool.tile(
                                [128, 512], F32, tag="gates", bufs=3,
                                name="gt")
                        # ar h-parts (shared AR weights)
                        for dd in (0, 1):
                            nc.tensor.matmul(g[("r", dd)][:], arw[:],
                                             ht[:, 512 * dd:512 * (dd + 1)],
                                             start=True, stop=False)
                        # ar x-parts (shared XR)
                        for dd in (0, 1):
                            nc.tensor.matmul(g[("r", dd)][:], w["XR"][:],
                                             xx[dd][:], start=False,
                                             stop=True)
                        rs = tpool.tile([128, DCH], BF16, tag="rs", bufs=2,
                                        name="rs")
                        for dd in (0, 1):
                            nc.scalar.activation(
                                rs[:, 512 * dd:512 * (dd + 1)],
                                g[("r", dd)][:], AF.Sigmoid)

                        g[("z", 0)] = pspool.tile([128, 512], F32,
                                                  tag="gates", bufs=3,
                                                  name="gt")
                        nc.tensor.matmul(g[("z", 0)][:], azw[:],
                                         ht[:, 0:512], start=True,
                                         stop=False)
                        nc.tensor.matmul(g[("z", 0)][:], w["XZ"][:],
                                         xx[0][:], start=False, stop=True)
                        # hn per-pair (separate allocation from np)
                        Ph = pspool.tile([128, DCH], F32, tag="P", bufs=2,
                                         name="Ph")
                        for dd in (0, 1):
                            nc.tensor.matmul(
                                Ph[:, 512 * dd:512 * (dd + 1)], w["WN"][:],
                                ht[:, 512 * dd:512 * (dd + 1)],
                                start=True, stop=True)
                        g[("z", 1)] = pspool.tile([128, 512], F32,
                                                  tag="gates", bufs=3,
                                                  name="gt")
                        nc.tensor.matmul(g[("z", 1)][:], azw[:],
                                         ht[:, 512:1024], start=True,
                                         stop=False)
                        nc.tensor.matmul(g[("z", 1)][:], w["XZ"][:],
                                         xx[1][:], start=False, stop=True)
                        zs = tpool.tile([128, DCH], BF16, tag="zs", bufs=3,
                                        name="zs")
                        for dd in (0, 1):
                            nc.scalar.activation(
                                zs[:, 512 * dd:512 * (dd + 1)],
                                g[("z", dd)][:], AF.Sigmoid)

                        # v = (hn + bnh) * r
                        vp = tpool.tile([128, DCH], BF16, tag="vp", bufs=2,
                                        name="vp")
                        for dd in (0, 1):
                            nc.vector.scalar_tensor_tensor(
                                vp[:, 512 * dd:512 * (dd + 1)],
                                Ph[:, 512 * dd:512 * (dd + 1)],
                                w["BNH"][:], rs[:, 512 * dd:512 * (dd + 1)],
                                op0=ALU.add, op1=ALU.mult)

                        # epoch evac (issued early so it overlaps flushes)
                        if state["pred_done"] is not None:
                            oldp, ep = state["pred_done"]
                            nc.vector.tensor_scalar_add(
                                outst[:, 512 * ep:512 * (ep + 1)],
                                oldp[:], _BUILT["bo"])
                            state["pred_done"] = None

                        # lag-2 pred flush
                        if len(state["pend"]) >= 2:
                            flush_blocks([state["pend"].pop(0)])

                        # np = Fn@h + Xn@x + I@v  (separate allocation)
                        Pn = pspool.tile([128, DCH], F32, tag="P", bufs=2,
                                         name="Pn")
                        if t > 0:
                            for dd in (0, 1):
                                nc.tensor.matmul(
                                    Pn[:, 512 * dd:512 * (dd + 1)],
                                    w["FN"][:],
                                    ht[:, 512 * dd:512 * (dd + 1)],
                                    start=True, stop=False)
                        for dd in (0, 1):
                            nc.tensor.matmul(
                                Pn[:, 512 * dd:512 * (dd + 1)],
                                w["XN"][:], xx[dd][:],
                                start=(t == 0), stop=True)
                        # np_tot = np_partial + v  (DVE; replaces the EYE pass)
                        npt = tpool.tile([128, DCH], BF16, tag="npt",
                                         bufs=2, name="npt")
                        nc.vector.tensor_tensor(npt[:], Pn[:], vp[:],
                                                op=ALU.add)
                        nt = tpool.tile([128, DCH], BF16, tag="nt", bufs=3,
                                        name="nt")
                        nc.scalar.activation(nt[:], npt[:], AF.Tanh)

                        # h'-update of the PREVIOUS pair (lag-1 issue)
                        issue_hprime()

                        hnew = hpool.tile([128, DCH], BF16, tag=f"h{pr}",
                                          bufs=3, name="hnew")
                        state["hprev"] = (ht, nt, zs, hnew)
                        hcur[pr] = hnew
                        state["pend"].append(
                            [(pr, 0, hnew, t % 8), (pr, 1, hnew, t % 8)])

                # epilogue: finish last pair's h', flush remaining preds,
                # evac final epoch, write out
                issue_hprime()
                if state["pend"]:
                    flush_blocks(state["pend"])
                state["pend"] = []
                nc.vector.tensor_scalar_add(
                    outst[:, 512 * (NEP - 1):512 * NEP],
                    state["pred"][:], _BUILT["bo"])
                nc.sync.dma_start(d_out[sg], outst[:])

    nc.compile()
    _BUILT["nc"] = nc
    return nc


def _prep_weights(W_in, b_in, W_ih, W_hh, b_ih, b_hh, W_out, b_out):
    import ml_dtypes
    f8 = np.float64
    G = W_ih.astype(f8) @ W_in.astype(f8)     # [192, 4]
    c = W_ih.astype(f8) @ b_in.astype(f8) + b_ih
    Wr, Wz, Wn = (W_hh[0:64].astype(f8), W_hh[64:128].astype(f8),
                  W_hh[128:192].astype(f8))
    brh, bzh, bnh = (b_hh[0:64].astype(f8), b_hh[64:128].astype(f8),
                     b_hh[128:192].astype(f8))
    cr, cz, cn = c[0:64], c[64:128], c[128:192]
    Gr0, Gz0, Gn0 = G[0:64, 0], G[64:128, 0], G[128:192, 0]
    Grx, Gzx, Gnx = G[0:64, 1:4], G[64:128, 1:4], G[128:192, 1:4]
    wo = W_out.astype(f8)[0]
    bo = float(b_out[0])

    Az = Wz + np.outer(Gz0, wo)
    Ar = Wr + np.outer(Gr0, wo)
    Fn = np.outer(Gn0, wo)
    dz0, dr0 = cz + bzh, cr + brh

    def bd(m):   # blockdiag of m.T ([64,64] -> [128,128] lhsT)
        out = np.zeros((128, 128), f8)
        out[0:64, 0:64] = m.T
        out[64:128, 64:128] = m.T
        return out

    def _wo32(wo, off):   # [128, 256]: 8 variants of [128,32] pred lhsT
        out = np.zeros((128, 256), f8)
        for t8 in range(8):
            out[0:64, 32 * t8 + off + 2 * t8] = wo
            out[64:128, 32 * t8 + off + 2 * t8 + 1] = wo
        return out

    def xw(Gx, G0, d):   # [12, 128] x-side lhsT
        blk = np.stack([Gx[:, 0], Gx[:, 1], Gx[:, 2], G0, d, G0 * bo],
                       axis=0)  # [6, 64]
        out = np.zeros((12, 128), f8)
        out[0:6, 0:64] = blk
        out[6:12, 64:128] = blk
        return out

    w = {
        "AZ1": bd(Az), "AR1": bd(Ar), "AZ0": bd(Wz), "AR0": bd(Wr),
        "WN": bd(Wn), "FN": bd(Fn), "EYE": np.eye(128),
        "XZ": xw(Gzx, Gz0, dz0), "XR": xw(Grx, Gr0, dr0),
        "XN": xw(Gnx, Gn0, cn),
        "WO32A": _wo32(wo, 0), "WO32B": _wo32(wo, 16),
        "BNH": np.concatenate([bnh, bnh])[:, None],
    }
    out = {}
    for k, v in w.items():
        dt = np.float32 if k == "BNH" else ml_dtypes.bfloat16
        out[k] = np.ascontiguousarray(v.astype(dt))
    _BUILT["bo"] = bo
    return out


def kernel(X, H, xn, W_in, b_in, W_ih, W_hh, b_ih, b_hh, W_out, b_out):
    global LAST_RESULTS
    import ml_dtypes
    X = np.asarray(X, np.float32)
    H = np.asarray(H, np.float32)
    xn = np.asarray(xn, np.float32)
    wmap = _prep_weights(np.asarray(W_in), np.asarray(b_in),
                         np.asarray(W_ih), np.asarray(W_hh),
                         np.asarray(b_ih), np.asarray(b_hh),
                         np.asarray(W_out), np.asarray(b_out))

    Xs = X[:, T_HIST:T_HIST + T_FC, :, F_IN - 3:F_IN]   # [B, 48, C, 3]

    in_maps = []
    for ci in range(N_CORES):
        bs = slice(ci * B_LOC, (ci + 1) * B_LOC)
        Xc = np.transpose(Xs[bs], (1, 0, 2, 3)).reshape(T_FC, NCOLS, 3)
        xnc = xn[bs, :, 0].reshape(NCOLS)
        Hc = H[bs].reshape(NCOLS, HID)

        HT = np.empty((NDCH, 128, 512), np.float32)
        XT = np.zeros((NDCH, T_FC, 12, 512), np.float32)
        for d in range(NDCH):
            for half in range(2):
                cs = slice(d * DCH + 512 * half, d * DCH + 512 * (half + 1))
                HT[d, 64 * half:64 * half + 64] = Hc[cs].T
                o = 6 * half
                XT[d, :, o:o + 3, :] = np.transpose(Xc[:, cs, :], (0, 2, 1))
                XT[d, 0, o + 3, :] = xnc[cs]       # xn row (t=0 only)
                XT[d, :, o + 4, :] = 1.0           # bias row
                XT[d, 1:, o + 5, :] = 1.0          # bo-step row (t>=1)
        m = {"HT": HT.astype(ml_dtypes.bfloat16),
             "XT": XT.astype(ml_dtypes.bfloat16)}
        m.update(wmap)
        in_maps.append(m)

    nc = _build()

    trace = os.environ.get("BASS_KERNEL_TRACE") == "1"
    if trace:
        _register_ntff_hook()
    res = run_bass_kernel_spmd(nc, in_maps, list(range(N_CORES)),
                               trace=trace)
    LAST_RESULTS = res

    out = np.empty((B, T_FC, C, 1), np.float32)
    t8 = np.arange(T_FC)
    for ci in range(N_CORES):
        O = res.results[ci]["OUT"].astype(np.float32)  # [NSG,128,512*NEP]
        O = O.reshape(NSG, 128, NEP, 512)
        core = np.empty((T_FC, NCOLS), np.float32)
        for sg in range(NSG):
            for dl in range(SG):
                d = sg * SG + dl
                for half in range(2):
                    cs = slice(d * DCH + 512 * half,
                               d * DCH + 512 * (half + 1))
                    core[:, cs] = O[sg,
                                    32 * (dl % 4) + 16 * (dl // 4)
                                    + 2 * (t8 % 8) + half, t8 // 8, :]
        bs = slice(ci * B_LOC, (ci + 1) * B_LOC)
        out[bs] = core.reshape(T_FC, B_LOC, C, 1).transpose(1, 0, 2, 3)
    return out


def _register_ntff_hook():
    import sys
    import types
    if "antenv.axon_hooks" in sys.modules:
        return
    mod = types.ModuleType("antenv.axon_hooks")
    state = {"hook": None}
    mod.set_axon_ntff_profile_hook = lambda h: state.update(hook=h)
    mod.get_axon_ntff_profile_hook = lambda: state["hook"]
    sys.modules["antenv.axon_hooks"] = mod
    try:
        import antenv
        antenv.axon_hooks = mod
    except ImportError:
        pass
    try:
        from trn_agent_boot.trn_boot import _ntff_profile_via_ctypes
        hook = _ntff_profile_via_ctypes("/opt/axon/libaxon_pjrt.so")
        if hook is not None:
            mod.set_axon_ntff_profile_hook(hook)
    except Exception as e:  # pragma: no cover
        print(f"NTFF hook registration failed: {e}")
    import concourse.bass_utils as bu
    bu.upload_artifacts = lambda tmpdir: f"file://{tmpdir}"


# revision 17
# speedup vs baseline: 1.6532x; 1.6532x over previous
"""Trainium2 Bass kernel for nn_Decoder — dual-column GRU decoder.

Design ("SG8v2") — zero-gap PE schedule
---------------------------------------
Data-parallel over batch: 8 cores x 8 batch rows -> 32768 columns per core.
A "dchunk" is 1024 columns stored as a [128, 512] tile — columns 0:512 on
partitions 0:64, columns 512:1024 on partitions 64:128.  Gate matmuls use
block-diagonal lhsT [128,128] = diag(W.T, W.T).

A supergroup of 8 dchunks = 4 pairs is pipelined per step.  The TRN2 tensor
engine only reaches 2.4 GHz after 3us of *continuous* execution, so the
whole schedule is built to never gap the PE:

- gate psum (az/ar): one shared tag, 3 banks, per-dchunk [128,512] tiles
  rotating ar-d0, ar-d1, az-d0, az-d1; the WAR distance to each sigmoid is
  >= 2 passes.
- hn and np live in SEPARATE per-pair [128,1024] allocations (tag P,
  bufs=2, 4 banks): fn/xn/eye never wait on the v-STT's read of hn.
- preds accumulate in ONE psum bank for all 8 dchunks
  (row = 32*pair + 16*(d%2) + 2*t8 + half) using two lhsT variants
  (WO32A/WO32B); flushes are lagged TWO pair-blocks; the h'-update DVE ops
  are lagged ONE pair-block so the vp STTs never queue behind them.
- epoch evacuation is issued at the head of pair-2's block of the next
  window, overlapping the new window's first flushes.

Per step t and pair (h [128,1024] = dchunks d0|d1):
  ar   = blockdiag(Ar.T) @ h + X_r @ xrows      (Ar = Wr + Gr0*wo' fold)
  az   = blockdiag(Az.T) @ h + X_z @ xrows
  r, z = sigmoid(ar), sigmoid(az)               (ACT, [128,512] per dchunk)
  hn   = blockdiag(Wn.T) @ h                    (per-pair PSUM)
  v    = (hn + b_hhn) * r                       (DVE STT per dchunk)
  np   = blockdiag(Fn.T)@h + X_n @ xrows        (separate per-pair PSUM)
  n    = tanh(np + v)                           (DVE add, then ACT
                                                 [128,1024]; no EYE pass)
  h'   = n + z*(h - n)                          (3 DVE ops, pair-wide,
                                                 issued in the next block)
  pred = wo-row lhsT @ h'  (+bo at evacuation), flushed two blocks later

x-rows per (dchunk, t): [12, 512] = per half [xt(3); xn(t=0); 1; bo-step].
"""

import os

import numpy as np

import concourse.bass as bass
import concourse.mybir as mybir
import concourse.tile as tile
from concourse import bacc
from concourse.bass_utils import run_bass_kernel_spmd

F32 = mybir.dt.float32
BF16 = mybir.dt.bfloat16
FP8 = mybir.dt.float8e4
AF = mybir.ActivationFunctionType
ALU = mybir.AluOpType

B, T_HIST, T_FC, C, F_IN, HID = 64, 24, 48, 4096, 8, 64
N_CORES = 8
B_LOC = B // N_CORES
NCOLS = B_LOC * C            # 32768 columns per core
DCH = 1024                   # columns per dchunk (dual-packed)
NDCH = NCOLS // DCH          # 32 dchunks
SG = 8                       # dchunks per supergroup
NSG = NDCH // SG             # 4 supergroups
NEP = T_FC // 8              # 6 pred epochs of 8 steps
XPF = 4                      # x prefetch lead (steps)

_BUILT = {}
LAST_RESULTS = None

W128 = ["AZ1", "AR1", "AZ0", "AR0", "WN", "FN", "EYE"]


def _build():
    if "nc" in _BUILT:
        return _BUILT["nc"]

    nc = bacc.Bacc("TRN2", target_bir_lowering=False, debug=False,
                   num_devices=N_CORES)

    d_ht = nc.dram_tensor("HT", [NDCH, 128, 512], BF16,
                          kind="ExternalInput").ap()
    d_xt = nc.dram_tensor("XT", [NDCH, T_FC, 12, 512], BF16,
                          kind="ExternalInput").ap()
    d_w = {}
    for name in W128:
        d_w[name] = nc.dram_tensor(name, [128, 128], BF16,
                                   kind="ExternalInput").ap()
    for name in ("XZ", "XR", "XN"):
        d_w[name] = nc.dram_tensor(name, [12, 128], BF16,
                                   kind="ExternalInput").ap()
    d_w["WO32A"] = nc.dram_tensor("WO32A", [128, 256], BF16,
                                  kind="ExternalInput").ap()
    d_w["WO32B"] = nc.dram_tensor("WO32B", [128, 256], BF16,
                                  kind="ExternalInput").ap()
    d_w["BNH"] = nc.dram_tensor("BNH", [128, 1], F32,
                                kind="ExternalInput").ap()
    d_out = nc.dram_tensor("OUT", [NSG, 128, 512 * NEP], BF16,
                           kind="ExternalOutput").ap()

    with tile.TileContext(nc) as tc:
        with (
            tc.tile_pool(name="wpool", bufs=1) as wpool,
            tc.tile_pool(name="xpool", bufs=1) as xpool,
            tc.tile_pool(name="hpool", bufs=1) as hpool,
            tc.tile_pool(name="tpool", bufs=1) as tpool,
            tc.tile_pool(name="opool", bufs=1) as opool,
            tc.tile_pool(name="pspool", bufs=1, space="PSUM") as pspool,
        ):
            w = {}
            for name, ap in d_w.items():
                wt = wpool.tile(list(ap.shape), ap.dtype, name=f"w_{name}")
                nc.gpsimd.dma_start(wt[:], ap[:])
                w[name] = wt

            hnext = None
            xts = {}

            def load_x(sgi, t0):
                for dp in range(SG):
                    xt = xpool.tile([12, 512], BF16, tag=f"x{dp}",
                                    bufs=XPF + 5, name="xt")
                    nc.sync.dma_start(xt[:], d_xt[sgi * SG + dp, t0])
                    xts[(sgi * SG + dp, t0)] = xt

            def load_h(sgi):
                hh = {}
                for pr in range(4):
                    ht = hpool.tile([128, DCH], BF16, tag=f"h{pr}", bufs=3,
                                    name="ht")
                    for dd in range(2):
                        d = sgi * SG + 2 * pr + dd
                        nc.sync.dma_start(ht[:, 512 * dd:512 * (dd + 1)],
                                          d_ht[d])
                    hh[pr] = ht
                return hh

            for sg in range(NSG):
                dbase = sg * SG
                hcur = hnext if hnext is not None else load_h(sg)
                hnext = None

                for t0 in range(XPF):
                    if (dbase, t0) not in xts:
                        load_x(sg, t0)

                outst = opool.tile([128, 512 * NEP], BF16, tag="ost",
                                   bufs=2, name="outstage")

                state = {"pend": [], "hprev": None, "pred": None,
                         "pred_done": None, "blk": 0}

                def flush_blocks(blks):
                    # blks: list of blocks, each [(pr, ddl, htile, t8), ...]
                    for blk in blks:
                        for (pr, ddl, htile, t8) in blk:
                            lhs = w["WO32A"] if ddl == 0 else w["WO32B"]
                            nc.tensor.matmul(
                                state["pred"][32 * pr:32 * pr + 32, :],
                                lhs[:, 32 * t8:32 * t8 + 32],
                                htile[:, 512 * ddl:512 * (ddl + 1)],
                                start=(t8 == 0 and ddl == 0),
                                stop=(t8 == 7 and ddl == 1),
                                tile_position=(0, 32 * pr))

                def issue_hprime():
                    hp = state["hprev"]
                    if hp is None:
                        return
                    ht_o, nt_o, zs_o, hnew_o = hp
                    hm = tpool.tile([128, DCH], BF16, tag="hm", bufs=2,
                                    name="hm")
                    nc.vector.tensor_tensor(hm[:], ht_o[:], nt_o[:],
                                            op=ALU.subtract)
                    ztt = tpool.tile([128, DCH], BF16, tag="zt", bufs=2,
                                     name="ztt")
                    nc.vector.tensor_tensor(ztt[:], zs_o[:], hm[:],
                                            op=ALU.mult)
                    nc.vector.tensor_tensor(hnew_o[:], nt_o[:], ztt[:],
                                            op=ALU.add)
                    state["hprev"] = None

                for t in range(T_FC):
                    if t == T_FC - 2 and sg + 1 < NSG:
                        hnext = load_h(sg + 1)
                        for t0 in range(XPF):
                            load_x(sg + 1, t0)
                    tp = t + XPF
                    if tp < T_FC:
                        load_x(sg, tp)

                    for pr in range(4):
                        ht = hcur[pr]
                        xx = {0: xts.pop((dbase + 2 * pr, t)),
                              1: xts.pop((dbase + 2 * pr + 1, t))}
                        azw = w["AZ1"] if t > 0 else w["AZ0"]
                        arw = w["AR1"] if t > 0 else w["AR0"]

                        # pred-window bookkeeping: at the 8-step boundary the
                        # evac of the finished window is issued at the head of
                        # pair-2's block, then the new pred bank is allocated.
                        if pr == 2 and t % 8 == 0:
                            if state["pred"] is not None:
                                state["pred_done"] = (state["pred"],
                                                      t // 8 - 1)
                            state["pred"] = pspool.tile(
                                [128, 512], F32, tag="pred", bufs=1,
                                name="predp")

                        # --- gate matmuls (3-bank rotation ar0,ar1,az0,az1)
                        g = {}
                        for nmr, dd in (("r", 0), ("r", 1)):
                            g[(nmr, dd)] = pspool.tile(
                                [128, 512], F32, tag="gates", bufs=3,
                                name="gt")
                        # ar h-parts (shared AR weights)
                        for dd in (0, 1):
                            nc.tensor.matmul(g[("r", dd)][:], arw[:],
                                             ht[:, 512 * dd:512 * (dd + 1)],
                                             start=True, stop=False)
                        # ar x-parts (shared XR)
                        for dd in (0, 1):
                            nc.tensor.matmul(g[("r", dd)][:], w["XR"][:],
                                             xx[dd][:], start=False,
                                             stop=True)
                        rs = tpool.tile([128, DCH], BF16, tag="rs", bufs=2,
                                        name="rs")
                        for dd in (0, 1):
                            nc.scalar.activation(
                                rs[:, 512 * dd:512 * (dd + 1)],
                                g[("r", dd)][:], AF.Sigmoid)

                        g[("z", 0)] = pspool.tile([128, 512], F32,
                                                  tag="gates", bufs=3,
                                                  name="gt")
                        nc.tensor.matmul(g[("z", 0)][:], azw[:],
                                         ht[:, 0:512], start=True,
                                         stop=False)
                        nc.tensor.matmul(g[("z", 0)][:], w["XZ"][:],
                                         xx[0][:], start=False, stop=True)
                        # hn per-pair (separate allocation from np)
                        Ph = pspool.tile([128, DCH], F32, tag="P", bufs=2,
                                         name="Ph")
                        for dd in (0, 1):
                            nc.tensor.matmul(
                                Ph[:, 512 * dd:512 * (dd + 1)], w["WN"][:],
                                ht[:, 512 * dd:512 * (dd + 1)],
                                start=True, stop=True)
                        g[("z", 1)] = pspool.tile([128, 512], F32,
                                                  tag="gates", bufs=3,
                                                  name="gt")
                        nc.tensor.matmul(g[("z", 1)][:], azw[:],
                                         ht[:, 512:1024], start=True,
                                         stop=False)
                        nc.tensor.matmul(g[("z", 1)][:], w["XZ"][:],
                                         xx[1][:], start=False, stop=True)
                        zs = tpool.tile([128, DCH], BF16, tag="zs", bufs=3,
                                        name="zs")
                        for dd in (0, 1):
                            nc.scalar.activation(
                                zs[:, 512 * dd:512 * (dd + 1)],
                                g[("z", dd)][:], AF.Sigmoid)

                        # v = (hn + bnh) * r
                        vp = tpool.tile([128, DCH], BF16, tag="vp", bufs=2,
                                        name="vp")
                        for dd in (0, 1):
                            nc.vector.scalar_tensor_tensor(
                                vp[:, 512 * dd:512 * (dd + 1)],
                                Ph[:, 512 * dd:512 * (dd + 1)],
                                w["BNH"][:], rs[:, 512 * dd:512 * (dd + 1)],
                                op0=ALU.add, op1=ALU.mult)

                        # epoch evac (issued early so it overlaps flushes)
                        if state["pred_done"] is not None:
                            oldp, ep = state["pred_done"]
                            nc.vector.tensor_scalar_add(
                                outst[:, 512 * ep:512 * (ep + 1)],
                                oldp[:], _BUILT["bo"])
                            state["pred_done"] = None

                        # lag-2 pred flush
                        if len(state["pend"]) >= 2:
                            flush_blocks([state["pend"].pop(0)])

                        # np = Fn@h + Xn@x + I@v  (separate allocation)
                        Pn = pspool.tile([128, DCH], F32, tag="P", bufs=2,
                                         name="Pn")
                        if t > 0:
                            for dd in (0, 1):
                                nc.tensor.matmul(
                                    Pn[:, 512 * dd:512 * (dd + 1)],
                                    w["FN"][:],
                                    ht[:, 512 * dd:512 * (dd + 1)],
                                    start=True, stop=False)
                        for dd in (0, 1):
                            nc.tensor.matmul(
                                Pn[:, 512 * dd:512 * (dd + 1)],
                                w["XN"][:], xx[dd][:],
                                start=(t == 0), stop=True)
                        # np_tot = np_partial + v  (DVE; replaces the EYE pass)
                        npt = tpool.tile([128, DCH], BF16, tag="npt",
                                         bufs=2, name="npt")
                        nc.vector.tensor_tensor(npt[:], Pn[:], vp[:],
                                                op=ALU.add)
                        nt = tpool.tile([128, DCH], BF16, tag="nt", bufs=3,
                                        name="nt")
                        nc.scalar.activation(nt[:], npt[:], AF.Tanh)

                        # h'-update of the PREVIOUS pair (lag-1 issue)
                        issue_hprime()

                        hnew = hpool.tile([128, DCH], BF16, tag=f"h{pr}",
                                          bufs=3, name="hnew")
                        state["hprev"] = (ht, nt, zs, hnew)
                        hcur[pr] = hnew
                        state["pend"].append(
                            [(pr, 0, hnew, t % 8), (pr, 1, hnew, t % 8)])

                # epilogue: finish last pair's h', flush remaining preds,
                # evac final epoch, write out
                issue_hprime()
                if state["pend"]:
                    flush_blocks(state["pend"])
                state["pend"] = []
                nc.vector.tensor_scalar_add(
                    outst[:, 512 * (NEP - 1):512 * NEP],
                    state["pred"][:], _BUILT["bo"])
                nc.sync.dma_start(d_out[sg], outst[:])

    nc.compile()
    _BUILT["nc"] = nc
    return nc


def _prep_weights(W_in, b_in, W_ih, W_hh, b_ih, b_hh, W_out, b_out):
    import ml_dtypes
    f8 = np.float64
    G = W_ih.astype(f8) @ W_in.astype(f8)     # [192, 4]
    c = W_ih.astype(f8) @ b_in.astype(f8) + b_ih
    Wr, Wz, Wn = (W_hh[0:64].astype(f8), W_hh[64:128].astype(f8),
                  W_hh[128:192].astype(f8))
    brh, bzh, bnh = (b_hh[0:64].astype(f8), b_hh[64:128].astype(f8),
                     b_hh[128:192].astype(f8))
    cr, cz, cn = c[0:64], c[64:128], c[128:192]
    Gr0, Gz0, Gn0 = G[0:64, 0], G[64:128, 0], G[128:192, 0]
    Grx, Gzx, Gnx = G[0:64, 1:4], G[64:128, 1:4], G[128:192, 1:4]
    wo = W_out.astype(f8)[0]
    bo = float(b_out[0])

    Az = Wz + np.outer(Gz0, wo)
    Ar = Wr + np.outer(Gr0, wo)
    Fn = np.outer(Gn0, wo)
    dz0, dr0 = cz + bzh, cr + brh

    def bd(m):   # blockdiag of m.T ([64,64] -> [128,128] lhsT)
        out = np.zeros((128, 128), f8)
        out[0:64, 0:64] = m.T
        out[64:128, 64:128] = m.T
        return out

    def _wo32(wo, off):   # [128, 256]: 8 variants of [128,32] pred lhsT
        out = np.zeros((128, 256), f8)
        for t8 in range(8):
            out[0:64, 32 * t8 + off + 2 * t8] = wo
            out[64:128, 32 * t8 + off + 2 * t8 + 1] = wo
        return out

    def xw(Gx, G0, d):   # [12, 128] x-side lhsT
        blk = np.stack([Gx[:, 0], Gx[:, 1], Gx[:, 2], G0, d, G0 * bo],
                       axis=0)  # [6, 64]
        out = np.zeros((12, 128), f8)
        out[0:6, 0:64] = blk
        out[6:12, 64:128] = blk
        return out

    w = {
        "AZ1": bd(Az), "AR1": bd(Ar), "AZ0": bd(Wz), "AR0": bd(Wr),
        "WN": bd(Wn), "FN": bd(Fn), "EYE": np.eye(128),
        "XZ": xw(Gzx, Gz0, dz0), "XR": xw(Grx, Gr0, dr0),
        "XN": xw(Gnx, Gn0, cn),
        "WO32A": _wo32(wo, 0), "WO32B": _wo32(wo, 16),
        "BNH": np.concatenate([bnh, bnh])[:, None],
    }
    out = {}
    for k, v in w.items():
        dt = np.float32 if k == "BNH" else ml_dtypes.bfloat16
        out[k] = np.ascontiguousarray(v.astype(dt))
    _BUILT["bo"] = bo
    return out


def kernel(X, H, xn, W_in, b_in, W_ih, W_hh, b_ih, b_hh, W_out, b_out):
    global LAST_RESULTS
    import ml_dtypes
    X = np.asarray(X, np.float32)
    H = np.asarray(H, np.float32)
    xn = np.asarray(xn, np.float32)
    wmap = _prep_weights(np.asarray(W_in), np.asarray(b_in),
                         np.asarray(W_ih), np.asarray(W_hh),
                         np.asarray(b_ih), np.asarray(b_hh),
                         np.asarray(W_out), np.asarray(b_out))

    Xs = X[:, T_HIST:T_HIST + T_FC, :, F_IN - 3:F_IN]   # [B, 48, C, 3]

    in_maps = []
    for ci in range(N_CORES):
        bs = slice(ci * B_LOC, (ci + 1) * B_LOC)
        Xc = np.transpose(Xs[bs], (1, 0, 2, 3)).reshape(T_FC, NCOLS, 3)
        xnc = xn[bs, :, 0].reshape(NCOLS)
        Hc = H[bs].reshape(NCOLS, HID)

        HT = np.empty((NDCH, 128, 512), np.float32)
        XT = np.zeros((NDCH, T_FC, 12, 512), np.float32)
        for d in range(NDCH):
            for half in range(2):
                cs = slice(d * DCH + 512 * half, d * DCH + 512 * (half + 1))
                HT[d, 64 * half:64 * half + 64] = Hc[cs].T
                o = 6 * half
                XT[d, :, o:o + 3, :] = np.transpose(Xc[:, cs, :], (0, 2, 1))
                XT[d, 0, o + 3, :] = xnc[cs]       # xn row (t=0 only)
                XT[d, :, o + 4, :] = 1.0           # bias row
                XT[d, 1:, o + 5, :] = 1.0          # bo-step row (t>=1)
        m = {"HT": HT.astype(ml_dtypes.bfloat16),
             "XT": XT.astype(ml_dtypes.bfloat16)}
        m.update(wmap)
        in_maps.append(m)

    nc = _build()

    trace = os.environ.get("BASS_KERNEL_TRACE") == "1"
    if trace:
        _register_ntff_hook()
    res = run_bass_kernel_spmd(nc, in_maps, list(range(N_CORES)),
                               trace=trace)
    LAST_RESULTS = res

    out = np.empty((B, T_FC, C, 1), np.float32)
    t8 = np.arange(T_FC)
    for ci in range(N_CORES):
        O = res.results[ci]["OUT"].astype(np.float32)  # [NSG,128,512*NEP]
        O = O.reshape(NSG, 128, NEP, 512)
        core = np.empty((T_FC, NCOLS), np.float32)
        for sg in range(NSG):
            for pr in range(4):
                for ddl in range(2):
                    d = sg * SG + 2 * pr + ddl
                    for half in range(2):
                        cs = slice(d * DCH + 512 * half,
                                   d * DCH + 512 * (half + 1))
                        core[:, cs] = O[sg,
                                        32 * pr + 16 * ddl + 2 * (t8 % 8)
                                        + half, t8 // 8, :]
        bs = slice(ci * B_LOC, (ci + 1) * B_LOC)
        out[bs] = core.reshape(T_FC, B_LOC, C, 1).transpose(1, 0, 2, 3)
    return out


def _register_ntff_hook():
    import sys
    import types
    if "antenv.axon_hooks" in sys.modules:
        return
    mod = types.ModuleType("antenv.axon_hooks")
    state = {"hook": None}
    mod.set_axon_ntff_profile_hook = lambda h: state.update(hook=h)
    mod.get_axon_ntff_profile_hook = lambda: state["hook"]
    sys.modules["antenv.axon_hooks"] = mod
    try:
        import antenv
        antenv.axon_hooks = mod
    except ImportError:
        pass
    try:
        from trn_agent_boot.trn_boot import _ntff_profile_via_ctypes
        hook = _ntff_profile_via_ctypes("/opt/axon/libaxon_pjrt.so")
        if hook is not None:
            mod.set_axon_ntff_profile_hook(hook)
    except Exception as e:  # pragma: no cover
        print(f"NTFF hook registration failed: {e}")
    import concourse.bass_utils as bu
    bu.upload_artifacts = lambda tmpdir: f"file://{tmpdir}"


# revision 18
# speedup vs baseline: 1.6620x; 1.0053x over previous
"""Trainium2 Bass kernel for nn_Decoder — dual-column GRU decoder.

Design ("SG8v2") — zero-gap PE schedule
---------------------------------------
Data-parallel over batch: 8 cores x 8 batch rows -> 32768 columns per core.
A "dchunk" is 1024 columns stored as a [128, 512] tile — columns 0:512 on
partitions 0:64, columns 512:1024 on partitions 64:128.  Gate matmuls use
block-diagonal lhsT [128,128] = diag(W.T, W.T).

A supergroup of 8 dchunks = 4 pairs is pipelined per step.  The TRN2 tensor
engine only reaches 2.4 GHz after 3us of *continuous* execution, so the
whole schedule is built to never gap the PE:

- gate psum (az/ar): one shared tag, 3 banks, per-dchunk [128,512] tiles
  rotating ar-d0, ar-d1, az-d0, az-d1; the WAR distance to each sigmoid is
  >= 2 passes.
- hn and np live in SEPARATE per-pair [128,1024] allocations (tag P,
  bufs=2, 4 banks): fn/xn/eye never wait on the v-STT's read of hn.
- preds accumulate in ONE psum bank for all 8 dchunks
  (row = 32*pair + 16*(d%2) + 2*t8 + half) using two lhsT variants
  (WO32A/WO32B); flushes are lagged TWO pair-blocks; the h'-update DVE ops
  are lagged ONE pair-block so the vp STTs never queue behind them.
- epoch evacuation is issued at the head of pair-2's block of the next
  window, overlapping the new window's first flushes.

Per step t and pair (h [128,1024] = dchunks d0|d1):
  ar   = blockdiag(Ar.T) @ h + X_r @ xrows      (Ar = Wr + Gr0*wo' fold)
  az   = blockdiag(Az.T) @ h + X_z @ xrows
  r, z = sigmoid(ar), sigmoid(az)               (ACT, [128,512] per dchunk)
  hn   = blockdiag(Wn.T) @ h                    (per-pair PSUM)
  v    = (hn + b_hhn) * r                       (DVE STT per dchunk)
  np   = blockdiag(Fn.T)@h + X_n @ xrows        (separate per-pair PSUM)
  n    = tanh(np + v)                           (DVE add, then ACT
                                                 [128,1024]; no EYE pass)
  h'   = n + z*(h - n)                          (3 DVE ops, pair-wide,
                                                 issued in the next block)
  pred = wo-row lhsT @ h'  (+bo at evacuation), flushed two blocks later

x-rows per (dchunk, t): [12, 512] = per half [xt(3); xn(t=0); 1; bo-step].
"""

import os

import numpy as np

import concourse.bass as bass
import concourse.mybir as mybir
import concourse.tile as tile
from concourse import bacc
from concourse.bass_utils import run_bass_kernel_spmd

F32 = mybir.dt.float32
BF16 = mybir.dt.bfloat16
FP8 = mybir.dt.float8e4
AF = mybir.ActivationFunctionType
ALU = mybir.AluOpType

B, T_HIST, T_FC, C, F_IN, HID = 64, 24, 48, 4096, 8, 64
N_CORES = 8
B_LOC = B // N_CORES
NCOLS = B_LOC * C            # 32768 columns per core
DCH = 1024                   # columns per dchunk (dual-packed)
NDCH = NCOLS // DCH          # 32 dchunks
SG = 8                       # dchunks per supergroup
NSG = NDCH // SG             # 4 supergroups
NEP = T_FC // 8              # 6 pred epochs of 8 steps
XPF = 4                      # x prefetch lead (steps)

_BUILT = {}
LAST_RESULTS = None

W128 = ["AZ1", "AR1", "AZ0", "AR0", "WN", "FN", "EYE"]


def _build():
    if "nc" in _BUILT:
        return _BUILT["nc"]

    nc = bacc.Bacc("TRN2", target_bir_lowering=False, debug=False,
                   num_devices=N_CORES)

    d_ht = nc.dram_tensor("HT", [NDCH, 128, 512], BF16,
                          kind="ExternalInput").ap()
    d_xt = nc.dram_tensor("XT", [NDCH, T_FC, 12, 512], BF16,
                          kind="ExternalInput").ap()
    d_w = {}
    for name in W128:
        d_w[name] = nc.dram_tensor(name, [128, 128], BF16,
                                   kind="ExternalInput").ap()
    for name in ("XZ", "XR", "XN"):
        d_w[name] = nc.dram_tensor(name, [12, 128], BF16,
                                   kind="ExternalInput").ap()
    d_w["WO32A"] = nc.dram_tensor("WO32A", [128, 256], BF16,
                                  kind="ExternalInput").ap()
    d_w["WO32B"] = nc.dram_tensor("WO32B", [128, 256], BF16,
                                  kind="ExternalInput").ap()
    d_w["BNH"] = nc.dram_tensor("BNH", [128, 1], F32,
                                kind="ExternalInput").ap()
    d_out = nc.dram_tensor("OUT", [NSG, 128, 512 * NEP], BF16,
                           kind="ExternalOutput").ap()

    with tile.TileContext(nc) as tc:
        with (
            tc.tile_pool(name="wpool", bufs=1) as wpool,
            tc.tile_pool(name="xpool", bufs=1) as xpool,
            tc.tile_pool(name="hpool", bufs=1) as hpool,
            tc.tile_pool(name="tpool", bufs=1) as tpool,
            tc.tile_pool(name="opool", bufs=1) as opool,
            tc.tile_pool(name="pspool", bufs=1, space="PSUM") as pspool,
        ):
            w = {}
            for name, ap in d_w.items():
                wt = wpool.tile(list(ap.shape), ap.dtype, name=f"w_{name}")
                nc.gpsimd.dma_start(wt[:], ap[:])
                w[name] = wt

            hnext = None
            xts = {}

            def load_x(sgi, t0):
                for dp in range(SG):
                    xt = xpool.tile([12, 512], BF16, tag=f"x{dp}",
                                    bufs=XPF + 5, name="xt")
                    nc.sync.dma_start(xt[:], d_xt[sgi * SG + dp, t0])
                    xts[(sgi * SG + dp, t0)] = xt

            def load_h(sgi):
                hh = {}
                for pr in range(4):
                    ht = hpool.tile([128, DCH], BF16, tag=f"h{pr}", bufs=3,
                                    name="ht")
                    for dd in range(2):
                        d = sgi * SG + 2 * pr + dd
                        nc.sync.dma_start(ht[:, 512 * dd:512 * (dd + 1)],
                                          d_ht[d])
                    hh[pr] = ht
                return hh

            for sg in range(NSG):
                dbase = sg * SG
                hcur = hnext if hnext is not None else load_h(sg)
                hnext = None

                for t0 in range(XPF):
                    if (dbase, t0) not in xts:
                        load_x(sg, t0)

                outst = opool.tile([128, 512 * NEP], BF16, tag="ost",
                                   bufs=2, name="outstage")

                state = {"pend": [], "hprev": None, "pred": None,
                         "pred_done": None, "blk": 0}

                def flush_blocks(blks):
                    # blks: list of blocks, each [(pr, ddl, htile, t8), ...]
                    for blk in blks:
                        for (pr, ddl, htile, t8) in blk:
                            lhs = w["WO32A"] if ddl == 0 else w["WO32B"]
                            nc.tensor.matmul(
                                state["pred"][32 * pr:32 * pr + 32, :],
                                lhs[:, 32 * t8:32 * t8 + 32],
                                htile[:, 512 * ddl:512 * (ddl + 1)],
                                start=(t8 == 0 and ddl == 0),
                                stop=(t8 == 7 and ddl == 1),
                                tile_position=(0, 32 * pr))

                def issue_hprime():
                    hp = state["hprev"]
                    if hp is None:
                        return
                    ht_o, nt_o, zs_o, hnew_o = hp
                    hm = tpool.tile([128, DCH], BF16, tag="hm", bufs=2,
                                    name="hm")
                    nc.vector.tensor_tensor(hm[:], ht_o[:], nt_o[:],
                                            op=ALU.subtract)
                    ztt = tpool.tile([128, DCH], BF16, tag="zt", bufs=2,
                                     name="ztt")
                    nc.vector.tensor_tensor(ztt[:], zs_o[:], hm[:],
                                            op=ALU.mult)
                    nc.vector.tensor_tensor(hnew_o[:], nt_o[:], ztt[:],
                                            op=ALU.add)
                    state["hprev"] = None

                for t in range(T_FC):
                    if t == T_FC - 2 and sg + 1 < NSG:
                        hnext = load_h(sg + 1)
                        for t0 in range(XPF):
                            load_x(sg + 1, t0)
                    tp = t + XPF
                    if tp < T_FC:
                        load_x(sg, tp)

                    for pr in range(4):
                        ht = hcur[pr]
                        xx = {0: xts.pop((dbase + 2 * pr, t)),
                              1: xts.pop((dbase + 2 * pr + 1, t))}
                        azw = w["AZ1"] if t > 0 else w["AZ0"]
                        arw = w["AR1"] if t > 0 else w["AR0"]

                        # pred-window bookkeeping: at the 8-step boundary the
                        # evac of the finished window is issued at the head of
                        # pair-2's block, then the new pred bank is allocated.
                        if pr == 2 and t % 8 == 0:
                            if state["pred"] is not None:
                                state["pred_done"] = (state["pred"],
                                                      t // 8 - 1)
                            state["pred"] = pspool.tile(
                                [128, 512], F32, tag="pred", bufs=1,
                                name="predp")

                        # --- gate matmuls (3-bank rotation ar0,ar1,az0,az1)
                        g = {}
                        for nmr, dd in (("r", 0), ("r", 1)):
                            g[(nmr, dd)] = pspool.tile(
                                [128, 512], F32, tag="gates", bufs=3,
                                name="gt")
                        # ar h-parts (shared AR weights)
                        for dd in (0, 1):
                            nc.tensor.matmul(g[("r", dd)][:], arw[:],
                                             ht[:, 512 * dd:512 * (dd + 1)],
                                             start=True, stop=False)
                        # ar x-parts (shared XR)
                        for dd in (0, 1):
                            nc.tensor.matmul(g[("r", dd)][:], w["XR"][:],
                                             xx[dd][:], start=False,
                                             stop=True)
                        rs = tpool.tile([128, DCH], BF16, tag="rs", bufs=2,
                                        name="rs")
                        for dd in (0, 1):
                            nc.scalar.activation(
                                rs[:, 512 * dd:512 * (dd + 1)],
                                g[("r", dd)][:], AF.Sigmoid)

                        g[("z", 0)] = pspool.tile([128, 512], F32,
                                                  tag="gates", bufs=3,
                                                  name="gt")
                        nc.tensor.matmul(g[("z", 0)][:], azw[:],
                                         ht[:, 0:512], start=True,
                                         stop=False)
                        nc.tensor.matmul(g[("z", 0)][:], w["XZ"][:],
                                         xx[0][:], start=False, stop=True)
                        # hn per-pair (separate allocation from np)
                        Ph = pspool.tile([128, DCH], F32, tag="P", bufs=2,
                                         name="Ph")
                        for dd in (0, 1):
                            nc.tensor.matmul(
                                Ph[:, 512 * dd:512 * (dd + 1)], w["WN"][:],
                                ht[:, 512 * dd:512 * (dd + 1)],
                                start=True, stop=True)
                        g[("z", 1)] = pspool.tile([128, 512], F32,
                                                  tag="gates", bufs=3,
                                                  name="gt")
                        nc.tensor.matmul(g[("z", 1)][:], azw[:],
                                         ht[:, 512:1024], start=True,
                                         stop=False)
                        nc.tensor.matmul(g[("z", 1)][:], w["XZ"][:],
                                         xx[1][:], start=False, stop=True)
                        zs = tpool.tile([128, DCH], BF16, tag="zs", bufs=3,
                                        name="zs")
                        for dd in (0, 1):
                            nc.scalar.activation(
                                zs[:, 512 * dd:512 * (dd + 1)],
                                g[("z", dd)][:], AF.Sigmoid)

                        # v = (hn + bnh) * r
                        vp = tpool.tile([128, DCH], BF16, tag="vp", bufs=2,
                                        name="vp")
                        for dd in (0, 1):
                            nc.vector.scalar_tensor_tensor(
                                vp[:, 512 * dd:512 * (dd + 1)],
                                Ph[:, 512 * dd:512 * (dd + 1)],
                                w["BNH"][:], rs[:, 512 * dd:512 * (dd + 1)],
                                op0=ALU.add, op1=ALU.mult)

                        # epoch evac (issued early so it overlaps flushes)
                        if state["pred_done"] is not None:
                            oldp, ep = state["pred_done"]
                            nc.vector.tensor_scalar_add(
                                outst[:, 512 * ep:512 * (ep + 1)],
                                oldp[:], _BUILT["bo"])
                            state["pred_done"] = None

                        # np = Fn@h + Xn@x + I@v  (separate allocation)
                        Pn = pspool.tile([128, DCH], F32, tag="P", bufs=2,
                                         name="Pn")
                        if t > 0:
                            for dd in (0, 1):
                                nc.tensor.matmul(
                                    Pn[:, 512 * dd:512 * (dd + 1)],
                                    w["FN"][:],
                                    ht[:, 512 * dd:512 * (dd + 1)],
                                    start=True, stop=False)
                        for dd in (0, 1):
                            nc.tensor.matmul(
                                Pn[:, 512 * dd:512 * (dd + 1)],
                                w["XN"][:], xx[dd][:],
                                start=(t == 0), stop=True)
                        # lag-2 pred flush at block end: max margin on the
                        # hnew write it reads
                        if len(state["pend"]) >= 2:
                            flush_blocks([state["pend"].pop(0)])
                        # np_tot = np_partial + v  (DVE; replaces the EYE pass)
                        npt = tpool.tile([128, DCH], BF16, tag="npt",
                                         bufs=2, name="npt")
                        nc.vector.tensor_tensor(npt[:], Pn[:], vp[:],
                                                op=ALU.add)
                        nt = tpool.tile([128, DCH], BF16, tag="nt", bufs=3,
                                        name="nt")
                        nc.scalar.activation(nt[:], npt[:], AF.Tanh)

                        # h'-update of the PREVIOUS pair (lag-1 issue)
                        issue_hprime()

                        hnew = hpool.tile([128, DCH], BF16, tag=f"h{pr}",
                                          bufs=3, name="hnew")
                        state["hprev"] = (ht, nt, zs, hnew)
                        hcur[pr] = hnew
                        state["pend"].append(
                            [(pr, 0, hnew, t % 8), (pr, 1, hnew, t % 8)])

                # epilogue: finish last pair's h', flush remaining preds,
                # evac final epoch, write out
                issue_hprime()
                if state["pend"]:
                    flush_blocks(state["pend"])
                state["pend"] = []
                nc.vector.tensor_scalar_add(
                    outst[:, 512 * (NEP - 1):512 * NEP],
                    state["pred"][:], _BUILT["bo"])
                nc.sync.dma_start(d_out[sg], outst[:])

    nc.compile()
    _BUILT["nc"] = nc
    return nc


def _prep_weights(W_in, b_in, W_ih, W_hh, b_ih, b_hh, W_out, b_out):
    import ml_dtypes
    f8 = np.float64
    G = W_ih.astype(f8) @ W_in.astype(f8)     # [192, 4]
    c = W_ih.astype(f8) @ b_in.astype(f8) + b_ih
    Wr, Wz, Wn = (W_hh[0:64].astype(f8), W_hh[64:128].astype(f8),
                  W_hh[128:192].astype(f8))
    brh, bzh, bnh = (b_hh[0:64].astype(f8), b_hh[64:128].astype(f8),
                     b_hh[128:192].astype(f8))
    cr, cz, cn = c[0:64], c[64:128], c[128:192]
    Gr0, Gz0, Gn0 = G[0:64, 0], G[64:128, 0], G[128:192, 0]
    Grx, Gzx, Gnx = G[0:64, 1:4], G[64:128, 1:4], G[128:192, 1:4]
    wo = W_out.astype(f8)[0]
    bo = float(b_out[0])

    Az = Wz + np.outer(Gz0, wo)
    Ar = Wr + np.outer(Gr0, wo)
    Fn = np.outer(Gn0, wo)
    dz0, dr0 = cz + bzh, cr + brh

    def bd(m):   # blockdiag of m.T ([64,64] -> [128,128] lhsT)
        out = np.zeros((128, 128), f8)
        out[0:64, 0:64] = m.T
        out[64:128, 64:128] = m.T
        return out

    def _wo32(wo, off):   # [128, 256]: 8 variants of [128,32] pred lhsT
        out = np.zeros((128, 256), f8)
        for t8 in range(8):
            out[0:64, 32 * t8 + off + 2 * t8] = wo
            out[64:128, 32 * t8 + off + 2 * t8 + 1] = wo
        return out

    def xw(Gx, G0, d):   # [12, 128] x-side lhsT
        blk = np.stack([Gx[:, 0], Gx[:, 1], Gx[:, 2], G0, d, G0 * bo],
                       axis=0)  # [6, 64]
        out = np.zeros((12, 128), f8)
        out[0:6, 0:64] = blk
        out[6:12, 64:128] = blk
        return out

    w = {
        "AZ1": bd(Az), "AR1": bd(Ar), "AZ0": bd(Wz), "AR0": bd(Wr),
        "WN": bd(Wn), "FN": bd(Fn), "EYE": np.eye(128),
        "XZ": xw(Gzx, Gz0, dz0), "XR": xw(Grx, Gr0, dr0),
        "XN": xw(Gnx, Gn0, cn),
        "WO32A": _wo32(wo, 0), "WO32B": _wo32(wo, 16),
        "BNH": np.concatenate([bnh, bnh])[:, None],
    }
    out = {}
    for k, v in w.items():
        dt = np.float32 if k == "BNH" else ml_dtypes.bfloat16
        out[k] = np.ascontiguousarray(v.astype(dt))
    _BUILT["bo"] = bo
    return out


def kernel(X, H, xn, W_in, b_in, W_ih, W_hh, b_ih, b_hh, W_out, b_out):
    global LAST_RESULTS
    import ml_dtypes
    X = np.asarray(X, np.float32)
    H = np.asarray(H, np.float32)
    xn = np.asarray(xn, np.float32)
    wmap = _prep_weights(np.asarray(W_in), np.asarray(b_in),
                         np.asarray(W_ih), np.asarray(W_hh),
                         np.asarray(b_ih), np.asarray(b_hh),
                         np.asarray(W_out), np.asarray(b_out))

    Xs = X[:, T_HIST:T_HIST + T_FC, :, F_IN - 3:F_IN]   # [B, 48, C, 3]

    in_maps = []
    for ci in range(N_CORES):
        bs = slice(ci * B_LOC, (ci + 1) * B_LOC)
        Xc = np.transpose(Xs[bs], (1, 0, 2, 3)).reshape(T_FC, NCOLS, 3)
        xnc = xn[bs, :, 0].reshape(NCOLS)
        Hc = H[bs].reshape(NCOLS, HID)

        HT = np.empty((NDCH, 128, 512), np.float32)
        XT = np.zeros((NDCH, T_FC, 12, 512), np.float32)
        for d in range(NDCH):
            for half in range(2):
                cs = slice(d * DCH + 512 * half, d * DCH + 512 * (half + 1))
                HT[d, 64 * half:64 * half + 64] = Hc[cs].T
                o = 6 * half
                XT[d, :, o:o + 3, :] = np.transpose(Xc[:, cs, :], (0, 2, 1))
                XT[d, 0, o + 3, :] = xnc[cs]       # xn row (t=0 only)
                XT[d, :, o + 4, :] = 1.0           # bias row
                XT[d, 1:, o + 5, :] = 1.0          # bo-step row (t>=1)
        m = {"HT": HT.astype(ml_dtypes.bfloat16),
             "XT": XT.astype(ml_dtypes.bfloat16)}
        m.update(wmap)
        in_maps.append(m)

    nc = _build()

    trace = os.environ.get("BASS_KERNEL_TRACE") == "1"
    if trace:
        _register_ntff_hook()
    res = run_bass_kernel_spmd(nc, in_maps, list(range(N_CORES)),
                               trace=trace)
    LAST_RESULTS = res

    out = np.empty((B, T_FC, C, 1), np.float32)
    t8 = np.arange(T_FC)
    for ci in range(N_CORES):
        O = res.results[ci]["OUT"].astype(np.float32)  # [NSG,128,512*NEP]
        O = O.reshape(NSG, 128, NEP, 512)
        core = np.empty((T_FC, NCOLS), np.float32)
        for sg in range(NSG):
            for pr in range(4):
                for ddl in range(2):
                    d = sg * SG + 2 * pr + ddl
                    for half in range(2):
                        cs = slice(d * DCH + 512 * half,
                                   d * DCH + 512 * (half + 1))
                        core[:, cs] = O[sg,
                                        32 * pr + 16 * ddl + 2 * (t8 % 8)
                                        + half, t8 // 8, :]
        bs = slice(ci * B_LOC, (ci + 1) * B_LOC)
        out[bs] = core.reshape(T_FC, B_LOC, C, 1).transpose(1, 0, 2, 3)
    return out


def _register_ntff_hook():
    import sys
    import types
    if "antenv.axon_hooks" in sys.modules:
        return
    mod = types.ModuleType("antenv.axon_hooks")
    state = {"hook": None}
    mod.set_axon_ntff_profile_hook = lambda h: state.update(hook=h)
    mod.get_axon_ntff_profile_hook = lambda: state["hook"]
    sys.modules["antenv.axon_hooks"] = mod
    try:
        import antenv
        antenv.axon_hooks = mod
    except ImportError:
        pass
    try:
        from trn_agent_boot.trn_boot import _ntff_profile_via_ctypes
        hook = _ntff_profile_via_ctypes("/opt/axon/libaxon_pjrt.so")
        if hook is not None:
            mod.set_axon_ntff_profile_hook(hook)
    except Exception as e:  # pragma: no cover
        print(f"NTFF hook registration failed: {e}")
    import concourse.bass_utils as bu
    bu.upload_artifacts = lambda tmpdir: f"file://{tmpdir}"
